# revision 25
# baseline (speedup 1.0000x reference)
"""Trainium2 Bass kernel for nn_Diagnet (S=1024, B=64, I=512, H=2048, O=512).

    u = einsum('sbi,hi->sbh', X, W_ih)
    h_t = |u_t + hh * h_{t-1}|   (scan over S, only final h needed)
    Y = h_final @ W_ho.T + b_ho

Strategy (8 NeuronCores, data-parallel over batch, 8 batch rows/core):

* H lanes permuted so hh is sorted descending, split into 16 chunks of
  128.  Chunk g only needs the last kg steps where amax(g)^kg ~ 1e-5
  (truncation, exact far below fp16 noise).  kg rounds up to 64-step
  blocks; chunks with kg == 64 are "shorts" (window = final block only).
* GEMM in fp16 (X, W_ih host-cast).  PSUM fp32, then the Activation
  engine copies each [128, (b,tau)] tile into a per-column fp16 u
  buffer.  X arrives in 4 large row-contiguous DMAs (block 15's tile
  first so the end-of-sequence work unblocks early).
* The scan runs on the DVE as a custom instruction ABS_SCAN_ANT:
      state_k = |state_{k-1} - u_k * scn_k|   (ABSOLUTE_DIFF prefix scan)
  with scn = NEGATED pre-scales -a^(K-1-t), so state_k tracks the
  pre-scaled recurrence m_t = a^(K-1-t) h_t and the final element IS
  h_final.  A mask (Idx >= K-1) + ADD-accum extracts the final state
  into m[:, (g,b)], which is also the s0 carry for the next piece of
  the same column.  One instruction covers up to a whole window.
* Shorts all merge into ONE scan stream per batch column: segments of
  [24 flush elements (POSITIVE scales 128*2^-j fold any state to
  <1e-5) + 64-step window].  Uniform 88-element segments put every
  chunk's final state at stride 88; one strided DVE copy gathers them
  into m.
* Block production order: 15 FIRST (it ends every window -> shorts and
  mid chunks unblock early), then the mid chunks' remaining blocks,
  then 0..11 ascending for chunk 0's piece-chasing.  The tail after
  the last GEMM is chunk 0's last piece + the final projection.
* Final projection: per chunk, m -> fp16 on the Activation engine,
  matmul vs fp16 W_ho^T accumulated in one PSUM bank, all issued at
  the very end (the PE runs in order - issuing them early would stall
  the PE queue on the DVE scan pipeline); bias added on DVE.
"""

import math
import os

from contextlib import ExitStack

import numpy as np

S, B, I, H, O = 1024, 64, 512, 2048, 512
NCORES = 8
BC = B // NCORES  # 8 batch rows per core
TB = 64  # time block
NBLK = S // TB  # 16
NCH = H // 128  # 16 h-chunks
NI = I // 128  # 4 i-chunks
XW = NI * TB * BC  # 2048 X cols per block (ic-major)
LN_TRUNC = 11.5  # a^K <= e^-11.5 ~ 1e-5 -> truncate (gate is 2e-2)
NFLUSH = 24  # 128*2^-24 ~ 7.6e-6 residual after flush
SEG = NFLUSH + TB  # 88-element short segment

_CACHE = {}


def _register_abs_scan():
    import concourse.dve_ops as dve_ops
    from concourse.dve_spec import Spec, Src0, Src1, Zero, C0, C1, scan, Idx, lower, AluOp
    from concourse.dve_uop import DveOpSpec

    for op in dve_ops.OPS:
        if op.name == "ABS_SCAN_ANT":
            return op

    def ref(in0, in1, s0, s1, imm2):
        x = in0.astype(np.float32) * in1.astype(np.float32)
        st = np.broadcast_to(np.asarray(s0, np.float32), x[:, 0].shape).copy()
        out = np.empty_like(x, dtype=np.float32)
        for k in range(x.shape[-1]):
            st = np.abs(st - x[:, k])
            out[:, k] = st * (k >= s1)
        return out

    state = scan(AluOp.ABSOLUTE_DIFF, Src0 * Src1, init=C0)
    spec = Spec(body=state * (Idx >= C1), accum=AluOp.ADD, accum_init=Zero, reference=ref)
    row = max(dve_ops._SUB_OPCODE_FOR_NAME.values()) + 1
    assert row < 0x20
    shas = {}
    for ver in ("v3", "v4"):
        s = DveOpSpec(name="ABS_SCAN_ANT", opcode=row, uops=lower(spec, ver=ver), rd1_en=True)
        shas[ver] = s.sha(ver)
    op = dve_ops.DveOp("ABS_SCAN_ANT", spec, subdim=False, uops_sha=shas)
    dve_ops._SUB_OPCODE_FOR_NAME["ABS_SCAN_ANT"] = row
    dve_ops.OPS.append(op)
    dve_ops.CUSTOM_DVE_SPECS["ABS_SCAN_ANT"] = spec
    return op


def _make_plan(hh):
    a = np.maximum(np.abs(hh.astype(np.float64)), 1e-30)
    perm = np.argsort(-a, kind="stable")
    a_s = a[perm]
    kgs = []
    for g in range(NCH):
        amax = a_s[g * 128]
        if amax >= math.exp(-LN_TRUNC / S):
            kg = S
        else:
            kg = min(S, int(math.ceil(LN_TRUNC / math.log(1.0 / amax))))
        kg = max(TB, min(S, ((kg + TB - 1) // TB) * TB))
        kgs.append(kg)
    assert all(kgs[g] >= kgs[g + 1] for g in range(NCH - 1)), kgs
    ag = a_s.reshape(NCH, 128)  # [chunk, lane]

    longs = [g for g in range(NCH) if kgs[g] > TB]
    shorts = [g for g in range(NCH) if kgs[g] == TB]
    NSH = len(shorts)

    # SCN layout: longs first (kg cols each, GENERATED ON-CHIP as a
    # geometric series out[t] = -a^kg * (1/a)^(t+1) = -a^(kg-1-t) via
    # gpsimd tensor_tensor_scan), then the merged shorts stream
    # (NSH segments of [NFLUSH flush + TB window], DMA'd).
    scn_off = {}
    off = 0
    for g in longs:
        scn_off[g] = off
        off += kgs[g]
    scn_shorts_off = off
    off += NSH * SEG
    scn_cols = off
    sh = np.zeros((128, max(NSH * SEG, 1)), dtype=np.float64)
    # flush elements fold |state - 128*2^-j| -> state collapses to <1e-5;
    # POSITIVE sign (the window scales are negated, these must not be).
    flush = 128.0 * (0.5 ** np.arange(NFLUSH))
    for i, g in enumerate(shorts):
        base = i * SEG
        sh[:, base : base + NFLUSH] = flush[None, :]
        t = np.arange(TB)
        sh[:, base + NFLUSH : base + SEG] = -(ag[g][:, None] ** (TB - 1 - t)[None, :])
    scn = sh.astype(np.float32)
    # per-long-chunk generator constants: 1/a and -a^kg
    aux = np.zeros((128, 2 * len(longs)), dtype=np.float64)
    for j, g in enumerate(longs):
        aux[:, 2 * j] = 1.0 / ag[g]
        aux[:, 2 * j + 1] = -(ag[g] ** kgs[g])
    aux = aux.astype(np.float32)

    # u layout: per long chunk g: BC columns of kg; then shorts: BC columns
    # of NSH*SEG.
    u_off = {}
    off = 0
    for g in longs:
        u_off[g] = off
        off += BC * kgs[g]
    u_shorts_off = off
    off += BC * NSH * SEG
    u_cols = off

    fb = {g: NBLK - kgs[g] // TB for g in longs}
    fb0 = fb[longs[0]]
    # block production order: 15 first, then remaining mid-chunk blocks
    # ascending, then chunk-0-only blocks ascending.
    mids = longs[1:]
    mid_lo = min((fb[g] for g in mids), default=NBLK - 1)
    order = [NBLK - 1]
    order += [kb for kb in range(mid_lo, NBLK - 1)]
    order += [kb for kb in range(fb0, mid_lo)]
    assert sorted(order) == list(range(fb0, NBLK)), (order, fb)

    # scan pieces: mids = one piece (their blocks all produced early);
    # chunk 0 split so pieces chase production, last piece covers the
    # late-produced blocks in one go.
    pieces = {}
    for g in mids:
        pieces[g] = [list(range(fb[g], NBLK))]
    nb0 = NBLK - fb0
    if nb0 <= 6:
        pieces[longs[0]] = [list(range(fb0, NBLK))]
    else:
        # e.g. fb0=0: [0-4], [5-9], [10-15]
        cut1 = fb0 + (nb0 - 6) // 2
        cut2 = fb0 + (nb0 - 6)
        grps = []
        if cut1 > fb0:
            grps.append(list(range(fb0, cut1)))
        if cut2 > cut1:
            grps.append(list(range(cut1, cut2)))
        grps.append(list(range(cut2, NBLK)))
        pieces[longs[0]] = grps

    return {
        "perm": perm,
        "kgs": tuple(kgs),
        "longs": tuple(longs),
        "shorts": tuple(shorts),
        "scn_off": scn_off,
        "scn_shorts_off": scn_shorts_off,
        "u_off": u_off,
        "u_shorts_off": u_shorts_off,
        "u_cols": u_cols,
        "fb": fb,
        "order": tuple(order),
        "pieces": pieces,
        "SCN": scn,
        "AUX": aux,
        "scn_cols": scn_cols,
    }


def _build(plan):
    import concourse.mybir as mybir
    import concourse.tile as tile
    from concourse import bacc
    from concourse.bass import ds

    ABS_SCAN = _register_abs_scan()
    f32 = mybir.dt.float32
    f16 = mybir.dt.float16

    kgs = plan["kgs"]
    longs = plan["longs"]
    shorts = plan["shorts"]
    NSH = len(shorts)
    scn_off = plan["scn_off"]
    scn_shorts_off = plan["scn_shorts_off"]
    u_off = plan["u_off"]
    u_shorts_off = plan["u_shorts_off"]
    u_cols = plan["u_cols"]
    fb = plan["fb"]
    order = plan["order"]
    pieces = plan["pieces"]
    scn_cols = plan["scn_cols"]
    NLG = len(longs)
    NORD = len(order)
    pos = {kb: i for i, kb in enumerate(order)}

    nc = bacc.Bacc("TRN2", target_bir_lowering=False, debug=False, num_devices=NCORES)
    # X rows are production-ordered: X[p, i, :] = block order[i], ic-major.
    X = nc.dram_tensor("X", [128, NORD, XW], f16, kind="ExternalInput").ap()
    WIHT = nc.dram_tensor("WIHT", [128, NI * H], f16, kind="ExternalInput").ap()
    WHOT = nc.dram_tensor("WHOT", [128, NCH * O], f16, kind="ExternalInput").ap()
    BIAS = nc.dram_tensor("BIAS", [BC, O], f32, kind="ExternalInput").ap()
    SCN = nc.dram_tensor("SCN", [128, plan["SCN"].shape[1]], f32, kind="ExternalInput").ap()
    AUX = nc.dram_tensor("AUX", [128, 2 * NLG], f32, kind="ExternalInput").ap()
    Y = nc.dram_tensor("Y", [BC, O], f32, kind="ExternalOutput").ap()

    # X DMA groups (by production position): [15] | next 3 | rest in three
    xgrp = [(0, 1), (1, min(4, NORD))]
    i = 4
    while i < NORD:
        j = min(i + 4, NORD)
        xgrp.append((i, j))
        i = j
    xgrp = [(a, b) for a, b in xgrp if b > a]

    with tile.TileContext(nc) as tc:
        with ExitStack() as ctx:
            consts = ctx.enter_context(tc.tile_pool(name="consts", bufs=1))
            gpool = ctx.enter_context(tc.tile_pool(name="gpsum", bufs=7, space="PSUM"))
            fpool = ctx.enter_context(tc.tile_pool(name="fpsum", bufs=1, space="PSUM"))

            # The Activation HW DGE queue's engines sustain ~330 GB/s under
            # compute; the SP queue's engines drop to ~90 GB/s.  Put all the
            # critical+bulk loads on the Act queue in need-time order; only
            # the end-needed WHOT/BIAS go on sync.
            wih_t = consts.tile([128, NI * H], f16, tag="wiht", name="wih_t")
            nc.scalar.dma_start(wih_t[:, ds(0, H)], WIHT[:, ds(0, H)])
            xb = {}
            xbt = []
            xdma = []
            for gi, (a, b) in enumerate(xgrp):
                t = consts.tile([128, (b - a) * XW], f16, tag=f"xb{gi}", name=f"xb{gi}")
                xdma.append((t, a, b))
                xbt.append(t)
                for p in range(a, b):
                    xb[order[p]] = (t, p - a)
            for ic in range(1, NI):
                nc.sync.dma_start(wih_t[:, ds(ic * H, H)], WIHT[:, ds(ic * H, H)])
            aux_t = consts.tile([128, 2 * NLG], f32, tag="aux", name="aux_t")
            nc.sync.dma_start(aux_t[:], AUX)
            bias_t = consts.tile([BC, O], f32, tag="bias", name="bias_t")
            nc.sync.dma_start(bias_t[:], BIAS)
            who_t = consts.tile([128, NCH * O], f16, tag="whot", name="who_t")
            nc.sync.dma_start(who_t[:], WHOT)
            t, a, b = xdma[0]
            nc.scalar.dma_start(t[:], X[:, ds(a, b - a), :])  # X-A (block 15)
            t, a, b = xdma[1]
            nc.scalar.dma_start(t[:], X[:, ds(a, b - a), :])  # X-B
            scn_t = consts.tile([128, scn_cols], f32, tag="scn", name="scn_t")
            if NSH:
                nc.scalar.dma_start(scn_t[:, ds(scn_shorts_off, NSH * SEG)], SCN)
            # X-C.. : alternate the two queues so neither starves the tail
            for qi, (t, a, b) in enumerate(xdma[2:]):
                (nc.scalar if qi % 2 == 0 else nc.sync).dma_start(
                    t[:], X[:, ds(a, b - a), :]
                )
            # generate the long chunks' scales on the idle GPSIMD engine:
            # scn[p, t] = -a^(kg-1-t) as a geometric series (ratio 1/a)
            zc = consts.tile([128, 1], f32, tag="zc", name="zc")
            nc.vector.memset(zc[:], 0.0)
            for j, g in enumerate(longs):
                kg = kgs[g]
                nc.vector.tensor_tensor_scan(
                    scn_t[:, ds(scn_off[g], kg)],
                    aux_t[:, ds(2 * j, 1)].broadcast_to([128, kg]),
                    zc[:].broadcast_to([128, kg]),
                    aux_t[:, ds(2 * j + 1, 1)],
                    mybir.AluOpType.mult,
                    mybir.AluOpType.add,
                )

            u_t = consts.tile([128, u_cols], f16, tag="u", name="u_t")
            m_t = consts.tile([128, NCH * BC], f32, tag="m", name="m_t")
            mh_t = consts.tile([128, NCH * BC], f16, tag="mh", name="mh_t")
            scr = consts.tile([128, max(NSH * SEG, S)], f32, tag="scr", name="scr")
            nc.vector.memset(m_t[:], 0.0)
            if NSH:
                # flush cells (u=1.0, scales carry the flush constants)
                fl = u_t[:, ds(u_shorts_off, BC * NSH * SEG)].rearrange(
                    "p (x s) -> p x s", x=BC * NSH, s=SEG
                )[:, :, ds(0, NFLUSH)]
                nc.gpsimd.memset(fl, 1.0)

            psy = fpool.tile([BC, O], f32, tag="fy", name="psy")

            def u3d(base_off, total, t):
                return u_t[:, ds(base_off, total)].rearrange(
                    "p (b t) -> p b t", b=BC, t=t
                )

            def copy_out(g, kb, src):
                # src: [p, b, tau] view of one block's GEMM result
                if g in shorts:
                    i = shorts.index(g)
                    dst = u3d(u_shorts_off, BC * NSH * SEG, NSH * SEG)[
                        :, :, ds(i * SEG + NFLUSH, TB)
                    ]
                else:
                    kg = kgs[g]
                    dst = u3d(u_off[g], BC * kg, kg)[:, :, ds((kb - fb[g]) * TB, TB)]
                nc.scalar.copy(dst, src)

            def produce(kb):
                # single-block production (block 15 and unpaired blocks)
                active = [g for g in longs if fb[g] <= kb]
                if kb == NBLK - 1:
                    active = active + list(shorts)
                xt, xi = xb[kb]
                # ic-OUTER within chunk groups: the first matmuls need only
                # WIHT-ic0, hiding the weight DMA latency.
                for lo in range(0, len(active), 7):
                    grp = active[lo : lo + 7]
                    ps = {
                        g: gpool.tile([128, TB * BC], f32, tag="gp", name=f"gp_{kb}_{g}")
                        for g in grp
                    }
                    for ic in range(NI):
                        for g in grp:
                            nc.tensor.matmul(
                                ps[g][:],
                                wih_t[:, ds(ic * H + g * 128, 128)],
                                xt[:, ds(xi * XW + ic * TB * BC, TB * BC)],
                                start=(ic == 0),
                                stop=(ic == NI - 1),
                            )
                    for g in grp:
                        copy_out(g, kb, ps[g][:].rearrange("p (b t) -> p b t", b=BC))

            def scan_piece(g, blks):
                kg = kgs[g]
                lo = (blks[0] - fb[g]) * TB
                n = len(blks) * TB
                for b in range(BC):
                    nc.vector._custom_dve(
                        ABS_SCAN,
                        out=scr[:, ds(0, n)],
                        in0=u_t[:, ds(u_off[g] + b * kg + lo, n)],
                        in1=scn_t[:, ds(scn_off[g] + lo, n)],
                        s0=m_t[:, ds(g * BC + b, 1)],
                        s1=float(n - 1),
                        accum_out=m_t[:, ds(g * BC + b, 1)],
                    )

            def scan_shorts():
                n = NSH * SEG
                for b in range(BC):
                    nc.vector._custom_dve(
                        ABS_SCAN,
                        out=scr[:, ds(0, n)],
                        in0=u_t[:, ds(u_shorts_off + b * n, n)],
                        in1=scn_t[:, ds(scn_shorts_off, n)],
                        s0=m_t[:, ds(shorts[0] * BC + b, 1)],
                        s1=float(-1.0),  # no mask; finals gathered from scr
                    )
                    src = scr[:, ds(0, n)].rearrange("p (s o) -> p s o", s=NSH, o=SEG)[
                        :, :, ds(SEG - 1, 1)
                    ]
                    dst = m_t[:, ds(shorts[0] * BC, NSH * BC)].rearrange(
                        "p (s o) -> p s o", s=NSH, o=BC
                    )[:, :, ds(b, 1)]
                    nc.vector.tensor_scalar_mul(dst, src, 1.0)

            # ---- schedule ----
            produced = []
            scanned_pieces = {g: 0 for g in longs}
            shorts_done = [False]

            def try_scans(at_end):
                if not shorts_done[0] and NBLK - 1 in produced and NSH:
                    scan_shorts()
                    shorts_done[0] = True
                for g in longs:
                    grps = pieces[g]
                    while scanned_pieces[g] < len(grps):
                        blks = grps[scanned_pieces[g]]
                        if not all(kb in produced for kb in blks):
                            break
                        # chunk 0's final piece is the tail; defer to the end
                        if (
                            g == longs[0]
                            and scanned_pieces[g] == len(grps) - 1
                            and not at_end
                        ):
                            break
                        scan_piece(g, blks)
                        scanned_pieces[g] += 1

            for kb in order:
                produce(kb)
                produced.append(kb)
                try_scans(False)
            try_scans(True)
            assert shorts_done[0] or not NSH
            assert all(scanned_pieces[g] == len(pieces[g]) for g in longs)

            # final projection, all deferred here (PE is in-order); the
            # chunk with the tail scan goes last.
            fin_order = list(longs[1:]) + list(shorts) + [longs[0]]
            for i, g in enumerate(fin_order):
                nc.scalar.copy(mh_t[:, ds(g * BC, BC)], m_t[:, ds(g * BC, BC)])
                nc.tensor.matmul(
                    psy[:],
                    mh_t[:, ds(g * BC, BC)],
                    who_t[:, ds(g * O, O)],
                    start=(i == 0),
                    stop=(i == NCH - 1),
                )

            y_t = consts.tile([BC, O], f32, tag="y", name="y_t")
            nc.vector.tensor_tensor(y_t[:], psy[:], bias_t[:], mybir.AluOpType.add)
            nc.scalar.dma_start(Y, y_t[:])  # fast queue; sync's crawls
    nc.compile()
    return nc


def _get_program(plan):
    key = (plan["kgs"], plan["longs"])
    if key not in _CACHE:
        _CACHE[key] = _build(plan)
    return _CACHE[key]


def _ensure_ntff_hook():
    """Provide antenv.axon_hooks (absent in this image) so trace=True works."""
    import sys
    import types

    if "antenv.axon_hooks" in sys.modules:
        return True
    try:
        import antenv

        mod = types.ModuleType("antenv.axon_hooks")
        mod._hook = None

        def set_axon_ntff_profile_hook(h):
            mod._hook = h

        def get_axon_ntff_profile_hook():
            return mod._hook

        mod.set_axon_ntff_profile_hook = set_axon_ntff_profile_hook
        mod.get_axon_ntff_profile_hook = get_axon_ntff_profile_hook
        sys.modules["antenv.axon_hooks"] = mod
        antenv.axon_hooks = mod

        from trn_agent_boot.trn_boot import _ntff_profile_via_ctypes

        hook = _ntff_profile_via_ctypes("/opt/axon/libaxon_pjrt.so")
        mod.set_axon_ntff_profile_hook(hook)
        return hook is not None
    except Exception:
        return False


def kernel(X, W_ih, hh, W_ho, b_ho):
    from concourse import bass_utils

    X = np.asarray(X, dtype=np.float32)
    W_ih = np.asarray(W_ih, dtype=np.float32)
    hh = np.asarray(hh, dtype=np.float32)
    W_ho = np.asarray(W_ho, dtype=np.float32)
    b_ho = np.asarray(b_ho, dtype=np.float32)

    plan = _make_plan(hh)
    perm = plan["perm"]
    order = plan["order"]
    nc = _get_program(plan)

    wiht = np.ascontiguousarray(W_ih[perm].T).astype(np.float16)  # [I, H]
    wiht = np.ascontiguousarray(
        wiht.reshape(NI, 128, H).transpose(1, 0, 2).reshape(128, NI * H)
    )
    whot = np.ascontiguousarray(W_ho[:, perm].T).astype(np.float16)  # [H, O]
    whot = np.ascontiguousarray(
        whot.reshape(NCH, 128, O).transpose(1, 0, 2).reshape(128, NCH * O)
    )
    bias = np.tile(b_ho[None, :], (BC, 1)).astype(np.float32)

    common = {
        "WIHT": wiht,
        "WHOT": whot,
        "BIAS": bias,
        "SCN": plan["SCN"],
        "AUX": plan["AUX"],
    }
    in_maps = []
    for m in range(NCORES):
        im = dict(common)
        xm = X[:, m * BC : (m + 1) * BC, :]  # [S, BC, I]
        # per block: [NI, 128, BC*TB] -> row-major [128, NI*TB*BC]
        xt = xm.transpose(2, 1, 0).reshape(NI, 128, BC, NBLK, TB)
        xt = xt.transpose(3, 1, 0, 2, 4).reshape(NBLK, 128, XW)
        xt = xt[list(order)]  # production order
        im["X"] = np.ascontiguousarray(xt.transpose(1, 0, 2)).astype(np.float16)
        in_maps.append(im)

    trace = bool(int(os.environ.get("DIAG_TRACE", "0")))
    if trace:
        trace = _ensure_ntff_hook()
    res = None
    for attempt in range(3):
        try:
            res = bass_utils.run_bass_kernel_spmd(
                nc,
                in_maps,
                core_ids=list(range(NCORES)),
                trace=trace,
                tmpdir=os.environ.get("DIAG_TRACE_DIR") or None,
            )
            break
        except Exception:
            if attempt == 2:
                raise
            trace = False  # retry without profiling
    if res.exec_time_ns is not None:
        kernel.last_exec_time_ns = res.exec_time_ns
        kernel.last_mean_exec_time_ns = res.mean_exec_time_ns
    Yfull = np.concatenate([r["Y"] for r in res.results], axis=0)
    return Yfull


kernel.last_exec_time_ns = None
kernel.last_mean_exec_time_ns = None


# revision 29
# speedup vs baseline: 1.0231x; 1.0231x over previous
"""Trainium2 Bass kernel for nn_Diagnet (S=1024, B=64, I=512, H=2048, O=512).

    u = einsum('sbi,hi->sbh', X, W_ih)
    h_t = |u_t + hh * h_{t-1}|   (scan over S, only final h needed)
    Y = h_final @ W_ho.T + b_ho

Strategy (8 NeuronCores, data-parallel over batch, 8 batch rows/core):

* H lanes permuted so hh is sorted descending, split into 16 chunks of
  128.  Chunk g only needs the last kg steps where amax(g)^kg ~ 1e-5
  (truncation, exact far below fp16 noise).  kg rounds up to 64-step
  blocks; chunks with kg == 64 are "shorts" (window = final block only).
* GEMM in fp16 (X, W_ih host-cast).  PSUM fp32, then the Activation
  engine copies each [128, (b,tau)] tile into a per-column fp16 u
  buffer.  X arrives in 4 large row-contiguous DMAs (block 15's tile
  first so the end-of-sequence work unblocks early).
* The scan runs on the DVE as a custom instruction ABS_SCAN_ANT:
      state_k = |state_{k-1} - u_k * scn_k|   (ABSOLUTE_DIFF prefix scan)
  with scn = NEGATED pre-scales -a^(K-1-t), so state_k tracks the
  pre-scaled recurrence m_t = a^(K-1-t) h_t and the final element IS
  h_final.  A mask (Idx >= K-1) + ADD-accum extracts the final state
  into m[:, (g,b)], which is also the s0 carry for the next piece of
  the same column.  One instruction covers up to a whole window.
* Shorts all merge into ONE scan stream per batch column: segments of
  [24 flush elements (POSITIVE scales 128*2^-j fold any state to
  <1e-5) + 64-step window].  Uniform 88-element segments put every
  chunk's final state at stride 88; one strided DVE copy gathers them
  into m.
* Block production order: 15 FIRST (it ends every window -> shorts and
  mid chunks unblock early), then the mid chunks' remaining blocks,
  then 0..11 ascending for chunk 0's piece-chasing.  The tail after
  the last GEMM is chunk 0's last piece + the final projection.
* Final projection: per chunk, m -> fp16 on the Activation engine,
  matmul vs fp16 W_ho^T accumulated in one PSUM bank, all issued at
  the very end (the PE runs in order - issuing them early would stall
  the PE queue on the DVE scan pipeline); bias added on DVE.
"""

import math
import os

from contextlib import ExitStack

import numpy as np

S, B, I, H, O = 1024, 64, 512, 2048, 512
NCORES = 8
BC = B // NCORES  # 8 batch rows per core
TB = 64  # time block
NBLK = S // TB  # 16
NCH = H // 128  # 16 h-chunks
NI = I // 128  # 4 i-chunks
XW = NI * TB * BC  # 2048 X cols per block (ic-major)
LN_TRUNC = 11.5  # a^K <= e^-11.5 ~ 1e-5 -> truncate (gate is 2e-2)
NFLUSH = 24  # 128*2^-24 ~ 7.6e-6 residual after flush
SEG = NFLUSH + TB  # 88-element short segment

_CACHE = {}


def _register_abs_scan():
    import concourse.dve_ops as dve_ops
    from concourse.dve_spec import Spec, Src0, Src1, Zero, C0, C1, scan, Idx, lower, AluOp
    from concourse.dve_uop import DveOpSpec

    for op in dve_ops.OPS:
        if op.name == "ABS_SCAN_ANT":
            return op

    def ref(in0, in1, s0, s1, imm2):
        x = in0.astype(np.float32) * in1.astype(np.float32)
        st = np.broadcast_to(np.asarray(s0, np.float32), x[:, 0].shape).copy()
        out = np.empty_like(x, dtype=np.float32)
        for k in range(x.shape[-1]):
            st = np.abs(st - x[:, k])
            out[:, k] = st * (k >= s1)
        return out

    state = scan(AluOp.ABSOLUTE_DIFF, Src0 * Src1, init=C0)
    spec = Spec(body=state * (Idx >= C1), accum=AluOp.ADD, accum_init=Zero, reference=ref)
    row = max(dve_ops._SUB_OPCODE_FOR_NAME.values()) + 1
    assert row < 0x20
    shas = {}
    for ver in ("v3", "v4"):
        s = DveOpSpec(name="ABS_SCAN_ANT", opcode=row, uops=lower(spec, ver=ver), rd1_en=True)
        shas[ver] = s.sha(ver)
    op = dve_ops.DveOp("ABS_SCAN_ANT", spec, subdim=False, uops_sha=shas)
    dve_ops._SUB_OPCODE_FOR_NAME["ABS_SCAN_ANT"] = row
    dve_ops.OPS.append(op)
    dve_ops.CUSTOM_DVE_SPECS["ABS_SCAN_ANT"] = spec
    return op


def _make_plan(hh):
    a = np.maximum(np.abs(hh.astype(np.float64)), 1e-30)
    perm = np.argsort(-a, kind="stable")
    a_s = a[perm]
    kgs = []
    for g in range(NCH):
        amax = a_s[g * 128]
        if amax >= math.exp(-LN_TRUNC / S):
            kg = S
        else:
            kg = min(S, int(math.ceil(LN_TRUNC / math.log(1.0 / amax))))
        kg = max(TB, min(S, ((kg + TB - 1) // TB) * TB))
        kgs.append(kg)
    assert all(kgs[g] >= kgs[g + 1] for g in range(NCH - 1)), kgs
    ag = a_s.reshape(NCH, 128)  # [chunk, lane]

    longs = [g for g in range(NCH) if kgs[g] > TB]
    shorts = [g for g in range(NCH) if kgs[g] == TB]
    NSH = len(shorts)

    # SCN layout: longs first (kg cols each, GENERATED ON-CHIP as a
    # geometric series out[t] = -a^kg * (1/a)^(t+1) = -a^(kg-1-t) via
    # gpsimd tensor_tensor_scan), then the merged shorts stream
    # (NSH segments of [NFLUSH flush + TB window], DMA'd).
    scn_off = {}
    off = 0
    for g in longs:
        scn_off[g] = off
        off += kgs[g]
    scn_shorts_off = off
    off += NSH * SEG
    scn_cols = off
    sh = np.zeros((128, max(NSH * SEG, 1)), dtype=np.float64)
    # flush elements fold |state - 128*2^-j| -> state collapses to <1e-5;
    # POSITIVE sign (the window scales are negated, these must not be).
    flush = 128.0 * (0.5 ** np.arange(NFLUSH))
    for i, g in enumerate(shorts):
        base = i * SEG
        sh[:, base : base + NFLUSH] = flush[None, :]
        t = np.arange(TB)
        sh[:, base + NFLUSH : base + SEG] = -(ag[g][:, None] ** (TB - 1 - t)[None, :])
    scn = sh.astype(np.float32)
    # per-long-chunk generator constants: 1/a and -a^kg
    aux = np.zeros((128, 2 * len(longs)), dtype=np.float64)
    for j, g in enumerate(longs):
        aux[:, 2 * j] = 1.0 / ag[g]
        aux[:, 2 * j + 1] = -(ag[g] ** kgs[g])
    aux = aux.astype(np.float32)

    # u layout: per long chunk g: BC columns of kg; then shorts: BC columns
    # of NSH*SEG.
    u_off = {}
    off = 0
    for g in longs:
        u_off[g] = off
        off += BC * kgs[g]
    u_shorts_off = off
    off += BC * NSH * SEG
    u_cols = off

    fb = {g: NBLK - kgs[g] // TB for g in longs}
    fb0 = fb[longs[0]]
    # block production order: 15 first, then remaining mid-chunk blocks
    # ascending, then chunk-0-only blocks ascending.
    mids = longs[1:]
    mid_lo = min((fb[g] for g in mids), default=NBLK - 1)
    order = [NBLK - 1]
    order += [kb for kb in range(mid_lo, NBLK - 1)]
    order += [kb for kb in range(fb0, mid_lo)]
    assert sorted(order) == list(range(fb0, NBLK)), (order, fb)

    # scan pieces: mids = one piece (their blocks all produced early);
    # chunk 0 split so pieces chase production, last piece covers the
    # late-produced blocks in one go.
    pieces = {}
    for g in mids:
        pieces[g] = [list(range(fb[g], NBLK))]
    nb0 = NBLK - fb0
    if nb0 <= 6:
        pieces[longs[0]] = [list(range(fb0, NBLK))]
    else:
        # e.g. fb0=0: [0-4], [5-9], [10-15]
        cut1 = fb0 + (nb0 - 6) // 2
        cut2 = fb0 + (nb0 - 6)
        grps = []
        if cut1 > fb0:
            grps.append(list(range(fb0, cut1)))
        if cut2 > cut1:
            grps.append(list(range(cut1, cut2)))
        grps.append(list(range(cut2, NBLK)))
        pieces[longs[0]] = grps

    return {
        "perm": perm,
        "kgs": tuple(kgs),
        "longs": tuple(longs),
        "shorts": tuple(shorts),
        "scn_off": scn_off,
        "scn_shorts_off": scn_shorts_off,
        "u_off": u_off,
        "u_shorts_off": u_shorts_off,
        "u_cols": u_cols,
        "fb": fb,
        "order": tuple(order),
        "pieces": pieces,
        "SCN": scn,
        "AUX": aux,
        "scn_cols": scn_cols,
    }


def _build(plan):
    import concourse.mybir as mybir
    import concourse.tile as tile
    from concourse import bacc
    from concourse.bass import ds

    ABS_SCAN = _register_abs_scan()
    f32 = mybir.dt.float32
    f16 = mybir.dt.float16

    kgs = plan["kgs"]
    longs = plan["longs"]
    shorts = plan["shorts"]
    NSH = len(shorts)
    scn_off = plan["scn_off"]
    scn_shorts_off = plan["scn_shorts_off"]
    u_off = plan["u_off"]
    u_shorts_off = plan["u_shorts_off"]
    u_cols = plan["u_cols"]
    fb = plan["fb"]
    order = plan["order"]
    pieces = plan["pieces"]
    scn_cols = plan["scn_cols"]
    NLG = len(longs)
    NORD = len(order)
    pos = {kb: i for i, kb in enumerate(order)}

    nc = bacc.Bacc("TRN2", target_bir_lowering=False, debug=False, num_devices=NCORES)
    # X rows are production-ordered: X[p, i, :] = block order[i], ic-major.
    X = nc.dram_tensor("X", [128, NORD, XW], f16, kind="ExternalInput").ap()
    WIHT = nc.dram_tensor("WIHT", [128, NI * H], f16, kind="ExternalInput").ap()
    WHOT = nc.dram_tensor("WHOT", [128, NCH * O], f16, kind="ExternalInput").ap()
    BIAS = nc.dram_tensor("BIAS", [BC, O], f32, kind="ExternalInput").ap()
    SCN = nc.dram_tensor("SCN", [128, plan["SCN"].shape[1]], f32, kind="ExternalInput").ap()
    AUX = nc.dram_tensor("AUX", [128, 2 * NLG], f32, kind="ExternalInput").ap()
    Y = nc.dram_tensor("Y", [BC, O], f32, kind="ExternalOutput").ap()

    # X DMA groups (by production position): [15] | next 3 | rest in three
    xgrp = [(0, 1), (1, min(4, NORD))]
    i = 4
    while i < NORD:
        j = min(i + 4, NORD)
        xgrp.append((i, j))
        i = j
    xgrp = [(a, b) for a, b in xgrp if b > a]

    with tile.TileContext(nc) as tc:
        with ExitStack() as ctx:
            consts = ctx.enter_context(tc.tile_pool(name="consts", bufs=1))
            gpool = ctx.enter_context(tc.tile_pool(name="gpsum", bufs=7, space="PSUM"))
            fpool = ctx.enter_context(tc.tile_pool(name="fpsum", bufs=1, space="PSUM"))

            # The Activation HW DGE queue's engines sustain ~330 GB/s under
            # compute; the SP queue's engines drop to ~90 GB/s.  Put all the
            # critical+bulk loads on the Act queue in need-time order; only
            # the end-needed WHOT/BIAS go on sync.
            wih_t = consts.tile([128, NI * H], f16, tag="wiht", name="wih_t")
            nc.scalar.dma_start(wih_t[:, ds(0, H // 2)], WIHT[:, ds(0, H // 2)])
            nc.scalar.dma_start(
                wih_t[:, ds(H // 2, H // 2)], WIHT[:, ds(H // 2, H // 2)]
            )
            xb = {}
            xbt = []
            xdma = []
            for gi, (a, b) in enumerate(xgrp):
                t = consts.tile([128, (b - a) * XW], f16, tag=f"xb{gi}", name=f"xb{gi}")
                xdma.append((t, a, b))
                xbt.append(t)
                for p in range(a, b):
                    xb[order[p]] = (t, p - a)
            aux_t = consts.tile([128, 2 * NLG], f32, tag="aux", name="aux_t")
            nc.sync.dma_start(aux_t[:], AUX)
            for ic in range(1, NI):
                nc.sync.dma_start(wih_t[:, ds(ic * H, H)], WIHT[:, ds(ic * H, H)])
            bias_t = consts.tile([BC, O], f32, tag="bias", name="bias_t")
            nc.sync.dma_start(bias_t[:], BIAS)
            who_t = consts.tile([128, NCH * O], f16, tag="whot", name="who_t")
            nc.sync.dma_start(who_t[:], WHOT)
            t, a, b = xdma[0]
            nc.scalar.dma_start(t[:], X[:, ds(a, b - a), :])  # X-A (block 15)
            t, a, b = xdma[1]
            nc.scalar.dma_start(t[:], X[:, ds(a, b - a), :])  # X-B
            scn_t = consts.tile([128, scn_cols], f32, tag="scn", name="scn_t")
            if NSH:
                nc.scalar.dma_start(scn_t[:, ds(scn_shorts_off, NSH * SEG)], SCN)
            # X-C..: bulk on the fast Act queue; sync only takes the final
            # group (needed last, and sync is the slower queue)
            rest = xdma[2:]
            for qi, (t, a, b) in enumerate(rest):
                q = nc.sync if qi == len(rest) - 1 else nc.scalar
                q.dma_start(t[:], X[:, ds(a, b - a), :])
            # generate the long chunks' scales on the idle GPSIMD engine:
            # scn[p, t] = -a^(kg-1-t) as a geometric series (ratio 1/a)
            zc = consts.tile([128, 1], f32, tag="zc", name="zc")
            nc.vector.memset(zc[:], 0.0)
            for j, g in enumerate(longs):
                kg = kgs[g]
                nc.vector.tensor_tensor_scan(
                    scn_t[:, ds(scn_off[g], kg)],
                    aux_t[:, ds(2 * j, 1)].broadcast_to([128, kg]),
                    zc[:].broadcast_to([128, kg]),
                    aux_t[:, ds(2 * j + 1, 1)],
                    mybir.AluOpType.mult,
                    mybir.AluOpType.add,
                )

            u_t = consts.tile([128, u_cols], f16, tag="u", name="u_t")
            m_t = consts.tile([128, NCH * BC], f32, tag="m", name="m_t")
            mh_t = consts.tile([128, NCH * BC], f16, tag="mh", name="mh_t")
            scr = consts.tile([128, max(NSH * SEG, S)], f32, tag="scr", name="scr")
            nc.vector.memset(m_t[:], 0.0)
            if NSH:
                # flush cells (u=1.0, scales carry the flush constants)
                fl = u_t[:, ds(u_shorts_off, BC * NSH * SEG)].rearrange(
                    "p (x s) -> p x s", x=BC * NSH, s=SEG
                )[:, :, ds(0, NFLUSH)]
                nc.gpsimd.memset(fl, 1.0)

            psy = fpool.tile([BC, O], f32, tag="fy", name="psy")

            def u3d(base_off, total, t):
                return u_t[:, ds(base_off, total)].rearrange(
                    "p (b t) -> p b t", b=BC, t=t
                )

            def copy_out(g, kb, src):
                # src: [p, b, tau] view of one block's GEMM result
                if g in shorts:
                    i = shorts.index(g)
                    dst = u3d(u_shorts_off, BC * NSH * SEG, NSH * SEG)[
                        :, :, ds(i * SEG + NFLUSH, TB)
                    ]
                else:
                    kg = kgs[g]
                    dst = u3d(u_off[g], BC * kg, kg)[:, :, ds((kb - fb[g]) * TB, TB)]
                nc.scalar.copy(dst, src)

            def produce(kb):
                active = [g for g in longs if fb[g] <= kb]
                if kb == NBLK - 1:
                    # shorts first: their copies unblock the merged scan
                    # stream (and then the mids) earliest
                    active = list(shorts) + active
                xt, xi = xb[kb]
                # ic-OUTER within chunk groups: the first matmuls need only
                # WIHT-ic0, hiding the weight DMA latency.
                for lo in range(0, len(active), 7):
                    grp = active[lo : lo + 7]
                    ps = {
                        g: gpool.tile([128, TB * BC], f32, tag="gp", name=f"gp_{kb}_{g}")
                        for g in grp
                    }
                    for ic in range(NI):
                        for g in grp:
                            nc.tensor.matmul(
                                ps[g][:],
                                wih_t[:, ds(ic * H + g * 128, 128)],
                                xt[:, ds(xi * XW + ic * TB * BC, TB * BC)],
                                start=(ic == 0),
                                stop=(ic == NI - 1),
                            )
                    for g in grp:
                        copy_out(g, kb, ps[g][:].rearrange("p (b t) -> p b t", b=BC))

            def scan_piece(g, blks):
                kg = kgs[g]
                lo = (blks[0] - fb[g]) * TB
                n = len(blks) * TB
                for b in range(BC):
                    nc.vector._custom_dve(
                        ABS_SCAN,
                        out=scr[:, ds(0, n)],
                        in0=u_t[:, ds(u_off[g] + b * kg + lo, n)],
                        in1=scn_t[:, ds(scn_off[g] + lo, n)],
                        s0=m_t[:, ds(g * BC + b, 1)],
                        s1=float(n - 1),
                        accum_out=m_t[:, ds(g * BC + b, 1)],
                    )

            def scan_shorts():
                n = NSH * SEG
                for b in range(BC):
                    nc.vector._custom_dve(
                        ABS_SCAN,
                        out=scr[:, ds(0, n)],
                        in0=u_t[:, ds(u_shorts_off + b * n, n)],
                        in1=scn_t[:, ds(scn_shorts_off, n)],
                        s0=m_t[:, ds(shorts[0] * BC + b, 1)],
                        s1=float(-1.0),  # no mask; finals gathered from scr
                    )
                    src = scr[:, ds(0, n)].rearrange("p (s o) -> p s o", s=NSH, o=SEG)[
                        :, :, ds(SEG - 1, 1)
                    ]
                    dst = m_t[:, ds(shorts[0] * BC, NSH * BC)].rearrange(
                        "p (s o) -> p s o", s=NSH, o=BC
                    )[:, :, ds(b, 1)]
                    nc.vector.tensor_scalar_mul(dst, src, 1.0)

            # ---- schedule ----
            produced = []
            scanned_pieces = {g: 0 for g in longs}
            shorts_done = [False]

            def try_scans(at_end):
                if not shorts_done[0] and NBLK - 1 in produced and NSH:
                    scan_shorts()
                    shorts_done[0] = True
                for g in longs:
                    grps = pieces[g]
                    while scanned_pieces[g] < len(grps):
                        blks = grps[scanned_pieces[g]]
                        if not all(kb in produced for kb in blks):
                            break
                        # chunk 0's final piece is the tail; defer to the end
                        if (
                            g == longs[0]
                            and scanned_pieces[g] == len(grps) - 1
                            and not at_end
                        ):
                            break
                        scan_piece(g, blks)
                        scanned_pieces[g] += 1

            for kb in order:
                produce(kb)
                produced.append(kb)
                try_scans(False)
            try_scans(True)
            assert shorts_done[0] or not NSH
            assert all(scanned_pieces[g] == len(pieces[g]) for g in longs)

            # final projection, all deferred here (PE is in-order); the
            # chunk with the tail scan goes last.
            fin_order = list(longs[1:]) + list(shorts) + [longs[0]]
            for i, g in enumerate(fin_order):
                nc.scalar.copy(mh_t[:, ds(g * BC, BC)], m_t[:, ds(g * BC, BC)])
                nc.tensor.matmul(
                    psy[:],
                    mh_t[:, ds(g * BC, BC)],
                    who_t[:, ds(g * O, O)],
                    start=(i == 0),
                    stop=(i == NCH - 1),
                )

            y_t = consts.tile([BC, O], f32, tag="y", name="y_t")
            nc.vector.tensor_tensor(y_t[:], psy[:], bias_t[:], mybir.AluOpType.add)
            nc.scalar.dma_start(Y, y_t[:])  # fast queue; sync's crawls
    nc.compile()
    return nc


def _get_program(plan):
    key = (plan["kgs"], plan["longs"])
    if key not in _CACHE:
        _CACHE[key] = _build(plan)
    return _CACHE[key]


def _ensure_ntff_hook():
    """Provide antenv.axon_hooks (absent in this image) so trace=True works."""
    import sys
    import types

    if "antenv.axon_hooks" in sys.modules:
        return True
    try:
        import antenv

        mod = types.ModuleType("antenv.axon_hooks")
        mod._hook = None

        def set_axon_ntff_profile_hook(h):
            mod._hook = h

        def get_axon_ntff_profile_hook():
            return mod._hook

        mod.set_axon_ntff_profile_hook = set_axon_ntff_profile_hook
        mod.get_axon_ntff_profile_hook = get_axon_ntff_profile_hook
        sys.modules["antenv.axon_hooks"] = mod
        antenv.axon_hooks = mod

        from trn_agent_boot.trn_boot import _ntff_profile_via_ctypes

        hook = _ntff_profile_via_ctypes("/opt/axon/libaxon_pjrt.so")
        mod.set_axon_ntff_profile_hook(hook)
        return hook is not None
    except Exception:
        return False


def kernel(X, W_ih, hh, W_ho, b_ho):
    from concourse import bass_utils

    X = np.asarray(X, dtype=np.float32)
    W_ih = np.asarray(W_ih, dtype=np.float32)
    hh = np.asarray(hh, dtype=np.float32)
    W_ho = np.asarray(W_ho, dtype=np.float32)
    b_ho = np.asarray(b_ho, dtype=np.float32)

    plan = _make_plan(hh)
    perm = plan["perm"]
    order = plan["order"]
    nc = _get_program(plan)

    wiht = np.ascontiguousarray(W_ih[perm].T).astype(np.float16)  # [I, H]
    wiht = np.ascontiguousarray(
        wiht.reshape(NI, 128, H).transpose(1, 0, 2).reshape(128, NI * H)
    )
    whot = np.ascontiguousarray(W_ho[:, perm].T).astype(np.float16)  # [H, O]
    whot = np.ascontiguousarray(
        whot.reshape(NCH, 128, O).transpose(1, 0, 2).reshape(128, NCH * O)
    )
    bias = np.tile(b_ho[None, :], (BC, 1)).astype(np.float32)

    common = {
        "WIHT": wiht,
        "WHOT": whot,
        "BIAS": bias,
        "SCN": plan["SCN"],
        "AUX": plan["AUX"],
    }
    in_maps = []
    for m in range(NCORES):
        im = dict(common)
        xm = X[:, m * BC : (m + 1) * BC, :]  # [S, BC, I]
        # per block: [NI, 128, BC*TB] -> row-major [128, NI*TB*BC]
        xt = xm.transpose(2, 1, 0).reshape(NI, 128, BC, NBLK, TB)
        xt = xt.transpose(3, 1, 0, 2, 4).reshape(NBLK, 128, XW)
        xt = xt[list(order)]  # production order
        im["X"] = np.ascontiguousarray(xt.transpose(1, 0, 2)).astype(np.float16)
        in_maps.append(im)

    trace = bool(int(os.environ.get("DIAG_TRACE", "0")))
    if trace:
        trace = _ensure_ntff_hook()
    res = None
    for attempt in range(3):
        try:
            res = bass_utils.run_bass_kernel_spmd(
                nc,
                in_maps,
                core_ids=list(range(NCORES)),
                trace=trace,
                tmpdir=os.environ.get("DIAG_TRACE_DIR") or None,
            )
            break
        except Exception:
            if attempt == 2:
                raise
            trace = False  # retry without profiling
    if res.exec_time_ns is not None:
        kernel.last_exec_time_ns = res.exec_time_ns
        kernel.last_mean_exec_time_ns = res.mean_exec_time_ns
    Yfull = np.concatenate([r["Y"] for r in res.results], axis=0)
    return Yfull


kernel.last_exec_time_ns = None
kernel.last_mean_exec_time_ns = None
